# revision 41
# baseline (speedup 1.0000x reference)
"""ChildSumTreeLSTMCell on 8 Trainium2 NeuronCores.

Strategy: sort edges by destination node on the host, partition nodes
contiguously across the 8 cores so every core's segment sums are fully
local -- zero collectives.  Edges are packed into subtiles (<=64 nodes,
<=256 edges = 2 chunks of 128 slots); 8 subtiles form a superblock
(512 node slots, 16 chunks, 2048 edge slots).  Segment sums are matmuls
against a 0/1 membership matrix M4 that is built on the HOST and shipped
as fp8 (64 node cols per chunk), so no on-device is_equal build.

v3 dtype plan (sim rel_err 0.0088): h fp8 + hw4 requantized fp8,
e/c/sfdf bf16, M4 fp8 (mixed-dtype matmuls vs bf16 lhsT), all PSUM f32.
DVE reads PSUM operands directly (no evacuation copies for ew/cs), gates
output bf16 so node-level elementwise runs in DVE 2x mode.
"""

import sys

for _p in ("/opt/trn_rl_repo", "/root/.axon_site/_ro/trn_rl_repo"):
    if _p not in sys.path:
        sys.path.append(_p)

import numpy as np
import ml_dtypes

import concourse.bacc as bacc
import concourse.mybir as mybir
import concourse.tile as tile
from concourse.bass_utils import run_bass_kernel_spmd

F32 = mybir.dt.float32
BF16 = mybir.dt.bfloat16
F8 = mybir.dt.float8e4

E = 500_000
N = 125_000
H = 128
G = 64
NCORES = 8
NPC = N // NCORES          # nodes per core
CHUNK = 128                # edge slots per chunk (contraction width)
SUB_N = 64                 # node slots per subtile
SUB_C = 2                  # chunks per subtile
SUB_E = SUB_C * CHUNK      # edge slots per subtile
SPB = 8                    # subtiles per superblock
BLK_C = SPB * SUB_C        # chunks per superblock (16)
BLK_E = SPB * SUB_E        # edge slots per superblock (2048)
BLK_N = SPB * SUB_N        # node slots per superblock (512)

bf16_np = ml_dtypes.bfloat16
f8_np = ml_dtypes.float8_e4m3

TRACE = False              # set by test.py to capture an NTFF profile
LAST = {}                  # last run's BassKernelResults


def _install_axon_hook():
    import types, contextlib, ctypes

    def _make_hook(so_path="/opt/axon/libaxon_pjrt.so"):
        lib = ctypes.CDLL(so_path)
        if not hasattr(lib, "axon_start_nrt_profile"):
            return None
        lib.axon_start_nrt_profile.argtypes = [
            ctypes.POINTER(ctypes.c_int64), ctypes.c_size_t]
        lib.axon_start_nrt_profile.restype = ctypes.c_int64
        lib.axon_stop_nrt_profile.argtypes = [ctypes.c_char_p]
        lib.axon_stop_nrt_profile.restype = ctypes.c_int64

        @contextlib.contextmanager
        def hook(output_dir, device_ids):
            import jax
            jax.devices()
            if device_ids:
                ids = (ctypes.c_int64 * len(device_ids))(*device_ids)
                rc = lib.axon_start_nrt_profile(ids, len(device_ids))
            else:
                rc = lib.axon_start_nrt_profile(None, 0)
            if rc != 0:
                raise RuntimeError("axon_start_nrt_profile rc=%d" % rc)
            try:
                yield
            finally:
                n = lib.axon_stop_nrt_profile(str(output_dir).encode())
                print("profile: %d file(s) written to %s" % (n, output_dir),
                      file=sys.stderr)

        return hook

    hook = _make_hook()
    mod = types.ModuleType("antenv.axon_hooks")
    mod.get_axon_ntff_profile_hook = lambda: hook
    mod.set_axon_ntff_profile_hook = lambda h: None
    sys.modules["antenv.axon_hooks"] = mod


def build_graph(SB):
    """Per-core Bass graph for SB superblocks."""
    nc = bacc.Bacc()
    assert SB % 2 == 0
    dp = nc.declare_dram_parameter
    d8_ext = dp("d8", [SB // 2, 128, 3 * BLK_E], F8, isOutput=False)
    d16_ext = dp("d16", [SB // 2, 128, 4 * BLK_E], BF16, isOutput=False)
    sfdf_ext = dp("sfdf", [SB // 2, G + 4, 4 * BLK_E], F8, isOutput=False)
    welT_ext = dp("welT", [G, 128], F32, isOutput=False)
    wa_ext = dp("wa", [G, 4], F32, isOutput=False)
    bel_ext = dp("belB", [4, 128], F32, isOutput=False)
    wg_ext = {}
    for x in "fiuo":
        wg_ext[x] = dp("w%sT" % x, [2 * H, 128], F32, isOutput=False)
    bias_ext = {}
    for x in "fiuo":
        bias_ext[x] = (dp("bW%s" % x, [H], F32, isOutput=False),
                       dp("b%s" % x, [H], F32, isOutput=False))
    out_ext = dp("outT", [128, SB * 2 * BLK_N], BF16, isOutput=True)

    AF = mybir.ActivationFunctionType
    D8W = 3 * BLK_E // 2   # per-sb cols in d8: h 2048 | M4 1024
    D16W = 2 * BLK_E       # per-sb cols in d16: e 2048 | c 2048
    SFW = 2 * BLK_E        # per-sb cols in sfdf: sf 2048 | df 2048

    with tile.TileContext(nc) as tc:
        cst = tc.alloc_tile_pool(name="cst", bufs=1)
        pin = tc.alloc_tile_pool(name="pin", bufs=4)
        pcv = tc.alloc_tile_pool(name="pcv", bufs=2)
        pnd = tc.alloc_tile_pool(name="pnd", bufs=4)
        pew = tc.alloc_tile_pool(name="pew", bufs=2, space="PSUM")
        pacc = tc.alloc_tile_pool(name="pacc", bufs=2, space="PSUM")
        pcs = tc.alloc_tile_pool(name="pcs", bufs=2, space="PSUM")
        pgp = pew

        # -- setup: constants -----------------------------------------------
        welT_sb = cst.tile([G, 128], F32)
        nc.sync.dma_start(out=welT_sb[:], in_=welT_ext[:])
        wa_sb = cst.tile([G, 4], F32)
        nc.sync.dma_start(out=wa_sb[:], in_=wa_ext[:])
        belr = cst.tile([4, 128], F32)
        nc.sync.dma_start(out=belr[:], in_=bel_ext[:])
        t2p = pgp.tile([4, 128], F32, tag="mm")
        nc.tensor.matmul(out=t2p[:], lhsT=wa_sb[:], rhs=welT_sb[:],
                         start=True, stop=True)
        wel_b16 = cst.tile([G, 128], BF16)
        nc.vector.tensor_copy(out=wel_b16[:], in_=welT_sb[:])
        t4b = cst.tile([4, 128], BF16)
        nc.vector.tensor_tensor(out=t4b[:], in0=t2p[:],
                                in1=belr[:], op=mybir.AluOpType.add)
        wtcomb = cst.tile([G + 4, 128], BF16)
        nc.sync.dma_start(out=wtcomb[0:G, :], in_=wel_b16[:])
        nc.sync.dma_start(out=wtcomb[G:G + 4, :], in_=t4b[:])

        wg = {}
        for x in "fiuo":
            stg = cst.tile([128, 128], F32, tag="wstg_%s" % x)
            nc.sync.dma_start(out=stg[:], in_=wg_ext[x][0:128, :])
            wa_t = cst.tile([128, 128], BF16, tag="wg_%s_a" % x)
            nc.vector.tensor_copy(out=wa_t[:], in_=stg[:])
            stg2 = cst.tile([128, 128], F32, tag="wstg2_%s" % x)
            nc.sync.dma_start(out=stg2[:], in_=wg_ext[x][128:256, :])
            wb_t = cst.tile([128, 128], BF16, tag="wg_%s_b" % x)
            nc.vector.tensor_copy(out=wb_t[:], in_=stg2[:])
            wg[x] = (wa_t, wb_t)

        bias = {}
        for x in "fiuo":
            b1 = cst.tile([128, 1], F32, tag="b1_%s" % x)
            nc.sync.dma_start(out=b1[:], in_=bias_ext[x][0][:, None])
            b2 = cst.tile([128, 1], F32, tag="b2_%s" % x)
            nc.sync.dma_start(out=b2[:], in_=bias_ext[x][1][:, None])
            bs = cst.tile([128, 1], F32, tag="bs_%s" % x)
            nc.vector.tensor_tensor(out=bs[:], in0=b1[:], in1=b2[:],
                                    op=mybir.AluOpType.add)
            bias[x] = bs

        # -- main loop: superblocks -----------------------------------------
        # Node-level assembly for superblock g is emitted during iteration
        # g+2 (at the top) so every input is long ready and no engine queue
        # blocks on a cross-engine dependency that isn't ready yet.
        hc2_ref = [None]

        def emit_assembly(g, gate, css):
            ct = pnd.tile([128, BLK_N], BF16, tag="ct")
            nc.vector.tensor_tensor(out=ct[:], in0=gate["f"][:],
                                    in1=css, op=mybir.AluOpType.mult)
            iu = pnd.tile([128, BLK_N], BF16, tag="iu")
            nc.gpsimd.tensor_tensor(out=iu[:], in0=gate["i"][:],
                                    in1=gate["u"][:], op=mybir.AluOpType.mult)
            if g % 2 == 0:
                hc2_ref[0] = pnd.tile([128, 4 * BLK_N], BF16, tag="hc2",
                                      name="hc2")
            hc = hc2_ref[0]
            o2 = (g % 2) * 2 * BLK_N
            nc.vector.tensor_tensor(out=hc[:, o2 + BLK_N:o2 + 2 * BLK_N],
                                    in0=iu[:], in1=ct[:],
                                    op=mybir.AluOpType.add)
            th = pnd.tile([128, BLK_N], BF16, tag="th")
            nc.scalar.activation(out=th[:], in_=hc[:, o2 + BLK_N:o2 + 2 * BLK_N],
                                 func=AF.Tanh)
            nc.vector.tensor_tensor(out=hc[:, o2:o2 + BLK_N], in0=gate["o"][:],
                                    in1=th[:], op=mybir.AluOpType.mult)
            if g % 2 == 1 or g == SB - 1:
                g0 = g - (g % 2)
                nc.gpsimd.dma_start(
                    out=out_ext[:, g0 * 2 * BLK_N:(g0 + 2) * 2 * BLK_N],
                    in_=hc[:])

        def emit_gates(g, hsab):
            gate = {}
            for x, fn in (("f", "Sigmoid"), ("i", "Sigmoid"),
                          ("u", "Tanh"), ("o", "Sigmoid")):
                gp = pgp.tile([128, BLK_N], F32, tag="mm")
                nc.tensor.matmul(out=gp[:], lhsT=wg[x][0][:],
                                 rhs=hsab[:, 0:BLK_N], start=True, stop=False)
                nc.tensor.matmul(out=gp[:], lhsT=wg[x][1][:],
                                 rhs=hsab[:, BLK_N:2 * BLK_N],
                                 start=False, stop=True)
                gs = pnd.tile([128, BLK_N], BF16, tag="g_%s" % x)
                nc.scalar.activation(out=gs[:], in_=gp[:],
                                     func=getattr(AF, fn), bias=bias[x][:])
                gate[x] = gs
            return gate

        def fetch_pair(p):
            d16 = pin.tile([128, 2 * D16W], BF16, tag="d16")
            nc.sync.dma_start(out=d16[:], in_=d16_ext[p])
            d8 = pin.tile([128, 2 * D8W], F8, tag="d8")
            nc.sync.dma_start(out=d8[:], in_=d8_ext[p])
            sfdf = pin.tile([G + 4, 2 * SFW], F8, tag="sfdf")
            nc.gpsimd.dma_start(out=sfdf[:], in_=sfdf_ext[p])
            return (d8, d16, sfdf)

        def emit_b68(g, pair):
            # B68 for superblock g, computed one iteration early so the ew
            # matmuls never wait on it (split DVE / Pool)
            sfdf = pair[2]
            j = (g % 2) * SFW
            Q = BLK_E // 2
            B68 = pcv.tile([G + 4, BLK_E], BF16, tag="B68")
            nc.vector.tensor_tensor(
                out=B68[:, 0:Q], in0=sfdf[:, j:j + Q],
                in1=sfdf[:, j + BLK_E:j + BLK_E + Q],
                op=mybir.AluOpType.mult)
            nc.gpsimd.tensor_tensor(
                out=B68[:, Q:BLK_E],
                in0=sfdf[:, j + Q:j + BLK_E],
                in1=sfdf[:, j + BLK_E + Q:j + 2 * BLK_E],
                op=mybir.AluOpType.mult)
            return B68

        evac = None     # (g, hsab, css) awaiting gate emission
        gated = None    # (g, gate, css) awaiting assembly emission
        pairs = {0: fetch_pair(0)}
        B68n = emit_b68(0, pairs[0])
        for g in range(SB):
            if g % 2 == 0 and g + 2 < SB:
                pairs[(g + 2) // 2] = fetch_pair((g + 2) // 2)
            pair = pairs[g // 2]
            d8p, d16p, _ = pair
            j8 = (g % 2) * D8W
            j16 = (g % 2) * D16W
            d8h = d8p[:, j8:j8 + BLK_E]                    # h (fp8)
            d8m = d8p[:, j8 + BLK_E:j8 + D8W]              # M4 (fp8)
            d16e = d16p[:, j16:j16 + BLK_E]                # e (bf16)
            d16c = d16p[:, j16 + BLK_E:j16 + D16W]         # c (bf16)

            B68 = B68n
            if g + 1 < SB:
                B68n = emit_b68(g + 1, pairs[(g + 1) // 2])

            # node assembly of superblock g-2: all inputs are long ready, so
            # these fill engine idle slots without blocking anything
            if gated is not None:
                emit_assembly(gated[0], gated[1], gated[2])
                gated = None

            # edge weights, hw4 = h * ew, and e/c segment sums, interleaved
            # so PE never idles while DVE computes hw4.
            hw4 = pcv.tile([128, BLK_E], BF16, tag="hw4")
            hs = pacc.tile([128, 2 * BLK_N], F32, tag="hs")
            cs = pcs.tile([128, BLK_N], F32, tag="cs")

            def ew_quarter(q):
                ew_ps = pew.tile([128, 512], F32, tag="mm")
                for c in range(4):
                    ch = q * 4 + c
                    nc.tensor.matmul(
                        out=ew_ps[:, c * 128:(c + 1) * 128],
                        lhsT=B68[:, ch * 128:(ch + 1) * 128],
                        rhs=wtcomb[:], start=True, stop=True)
                nc.vector.tensor_tensor(
                    out=hw4[:, q * 512:(q + 1) * 512],
                    in0=d8h[:, q * 512:(q + 1) * 512], in1=ew_ps[:],
                    op=mybir.AluOpType.mult)

            def ec_segsum(st_lo, st_hi):
                for st in range(st_lo, st_hi):
                    for k in range(SUB_C):
                        ch = st * SUB_C + k
                        m4 = d8m[:, ch * SUB_N:(ch + 1) * SUB_N]
                        nc.tensor.matmul(
                            out=hs[:, BLK_N + st * SUB_N:BLK_N + (st + 1) * SUB_N],
                            lhsT=d16e[:, ch * 128:(ch + 1) * 128],
                            rhs=m4, start=(k == 0), stop=(k == SUB_C - 1))
                        nc.tensor.matmul(
                            out=cs[:, st * SUB_N:(st + 1) * SUB_N],
                            lhsT=d16c[:, ch * 128:(ch + 1) * 128],
                            rhs=m4, start=(k == 0), stop=(k == SUB_C - 1))

            ew_quarter(0)
            ew_quarter(1)
            ec_segsum(0, 4)
            ew_quarter(2)
            ew_quarter(3)
            ec_segsum(4, 8)

            # gates of the previous superblock: fills PE/ACT while this
            # superblock's hw4 finishes on DVE
            if evac is not None:
                gated = (evac[0], emit_gates(evac[0], evac[1]), evac[2])

            for st in range(SPB):
                for k in range(SUB_C):
                    ch = st * SUB_C + k
                    m4 = d8m[:, ch * SUB_N:(ch + 1) * SUB_N]
                    nc.tensor.matmul(
                        out=hs[:, st * SUB_N:(st + 1) * SUB_N],
                        lhsT=hw4[:, ch * 128:(ch + 1) * 128],
                        rhs=m4, start=(k == 0), stop=(k == SUB_C - 1))

            # evacuate PSUM; cols [hs_h 512 | hs_e 512 | cs 512]
            hsx = pnd.tile([128, 3 * BLK_N], BF16, tag="hsx")
            nc.scalar.activation(out=hsx[:, 0:2 * BLK_N], in_=hs[:],
                                 func=AF.Copy)
            nc.scalar.activation(out=hsx[:, 2 * BLK_N:3 * BLK_N], in_=cs[:],
                                 func=AF.Copy)
            evac = (g, hsx, hsx[:, 2 * BLK_N:3 * BLK_N])

        if gated is not None:
            emit_assembly(gated[0], gated[1], gated[2])
        gated = (evac[0], emit_gates(evac[0], evac[1]), evac[2])
        emit_assembly(gated[0], gated[1], gated[2])

        for p in (pcs, pacc, pew, pnd, pcv, pin, cst):
            p.release()
    nc.finalize()
    return nc


def plan_subtiles(dst_local, npc):
    """Greedy: <=SUB_N nodes and <=SUB_E edges per subtile.
    Returns list of (n0, n1, e0, e1) using sorted-edge offsets."""
    cnt = np.bincount(dst_local, minlength=npc)
    cum = np.concatenate([[0], np.cumsum(cnt)])
    tiles = []
    s = 0
    while s < npc:
        hi = min(s + SUB_N, npc)
        m = int(np.searchsorted(cum, cum[s] + SUB_E, side="right")) - 1
        m = max(s + 1, min(hi, m))
        tiles.append((s, m, int(cum[s]), int(cum[m])))
        s = m
    return tiles


def prep_core(k, h_src, c_src, embed_dst, src_f, dst_f, etype, dst, SB):
    """Build one core's padded superblock arrays."""
    lo = k * NPC
    sel = np.nonzero((dst >= lo) & (dst < lo + NPC))[0]
    dl = (dst[sel] - lo).astype(np.int64)
    order = np.argsort(dl, kind="stable")
    eidx = sel[order]
    dls = dl[order]
    tiles = plan_subtiles(dls, NPC)
    T = SB * SPB
    assert len(tiles) <= T
    ES = T * SUB_E
    src_slot = np.full(ES, -1, dtype=np.int64)
    nl_slot = np.zeros(ES, dtype=np.int64)      # node idx within subtile
    for t, (n0, n1, e0, e1) in enumerate(tiles):
        ne = e1 - e0
        assert ne <= SUB_E and n1 - n0 <= SUB_N
        src_slot[t * SUB_E:t * SUB_E + ne] = eidx[e0:e1]
        nl_slot[t * SUB_E:t * SUB_E + ne] = dls[e0:e1] - n0
    val = src_slot >= 0
    gi = src_slot[val]

    def pad_rows(a, w):
        out = np.zeros((ES, w), dtype=np.float32)
        out[val] = a[gi]
        return out

    def chunk_layout(a, w):
        # [ES, w] -> [SB, 128, BLK_C*w]: slot (sb, ch, p) dim d at
        # [sb, p, ch*w + d]
        return np.ascontiguousarray(
            a.reshape(SB, BLK_C, CHUNK, w).transpose(0, 2, 1, 3)
             .reshape(SB, 128, BLK_C * w))

    # membership: [sb, p, ch*64 + j] = (nl_slot of (sb,ch,p) == j)
    nl = nl_slot.reshape(SB, BLK_C, CHUNK)
    vl = val.reshape(SB, BLK_C, CHUNK)
    m4 = (nl[:, :, :, None] == np.arange(SUB_N)[None, None, None, :])
    m4 = (m4 & vl[:, :, :, None]).astype(np.float32)
    m4 = m4.reshape(SB, BLK_C, CHUNK, SUB_N).transpose(0, 2, 1, 3) \
           .reshape(SB, 128, BLK_C * SUB_N)

    def pair(a):
        # [SB, P, W] -> [SB/2, P, 2W]
        S, P, W = a.shape
        return np.ascontiguousarray(
            a.reshape(S // 2, 2, P, W).transpose(0, 2, 1, 3)
             .reshape(S // 2, P, 2 * W))

    h8 = chunk_layout(pad_rows(h_src, H), H)
    d8 = pair(np.concatenate([h8, m4], axis=2)).astype(f8_np)

    ep = chunk_layout(pad_rows(embed_dst, H), H)
    cp = chunk_layout(pad_rows(c_src, H), H)
    d16 = pair(np.concatenate([ep, cp], axis=2)).astype(bf16_np)

    # sf' = [sf | onehot4], df' = [df | ones]: [SB, 68, ch*128 + p]
    sfp = np.zeros((ES, G + 4), dtype=np.float32)
    sfp[val, :G] = src_f[gi]
    sfp[val, G + etype[gi]] = 1.0
    sfp[val, G + 3] = 1.0
    dfp = np.zeros((ES, G + 4), dtype=np.float32)
    dfp[val, :G] = dst_f[gi]
    dfp[val, G:] = 1.0
    def feat_layout(a):
        return a.reshape(SB, BLK_C * CHUNK, G + 4).transpose(0, 2, 1)
    sfdf = pair(np.concatenate(
        [feat_layout(sfp), feat_layout(dfp)], axis=2)).astype(f8_np)

    return {"d8": d8, "d16": d16, "sfdf": sfdf}, tiles


_graph_cache = {}


def kernel(**inputs):
    h_src = np.asarray(inputs["h_src"], dtype=np.float32)
    c_src = np.asarray(inputs["c_src"], dtype=np.float32)
    embed_dst = np.asarray(inputs["embed_dst"], dtype=np.float32)
    src_f = np.asarray(inputs["src_node_feat"], dtype=np.float32)
    dst_f = np.asarray(inputs["dst_node_feat"], dtype=np.float32)
    etype = np.asarray(inputs["edge_type_idx"]).astype(np.int64)
    dst = np.asarray(inputs["dst_idx"]).astype(np.int64)

    weights = {
        "welT": np.ascontiguousarray(np.asarray(inputs["W_el"], np.float32).T),
        "wa": np.ascontiguousarray(np.concatenate(
            [np.asarray(inputs["W_eoh"], np.float32),
             np.asarray(inputs["b_eoh"], np.float32)[:, None]], axis=1)),
    }
    belB = np.zeros((4, 128), dtype=np.float32)
    belB[3] = np.asarray(inputs["b_el"], np.float32)
    weights["belB"] = belB
    for x, wn, bwn, bn in (("f", "Wf", "bWf", "bf"), ("i", "Wi", "bWi", "bi"),
                           ("u", "Wu", "bWu", "bu"), ("o", "Wo", "bWo", "bo")):
        weights["w%sT" % x] = np.ascontiguousarray(
            np.asarray(inputs[wn], np.float32).T)
        weights["bW%s" % x] = np.asarray(inputs[bwn], np.float32)
        weights["b%s" % x] = np.asarray(inputs[bn], np.float32)

    planned = []
    for k in range(NCORES):
        lo = k * NPC
        sel = np.nonzero((dst >= lo) & (dst < lo + NPC))[0]
        dl = np.sort((dst[sel] - lo).astype(np.int64))
        planned.append(plan_subtiles(dl, NPC))
    T = max(len(p) for p in planned)
    SB = (T + SPB - 1) // SPB
    SB += SB % 2

    in_maps = []
    tiles_all = []
    for k in range(NCORES):
        m, tiles = prep_core(k, h_src, c_src, embed_dst, src_f, dst_f,
                             etype, dst, SB)
        m.update(weights)
        in_maps.append(m)
        tiles_all.append(tiles)

    if SB not in _graph_cache:
        _graph_cache[SB] = build_graph(SB)
    nc = _graph_cache[SB]

    if TRACE:
        _install_axon_hook()
    res = run_bass_kernel_spmd(nc, in_maps, list(range(NCORES)), trace=TRACE)
    LAST["res"] = res

    out = np.empty((N, 2 * H), dtype=np.float32)
    for k in range(NCORES):
        outT = np.asarray(res.results[k]["outT"]).astype(np.float32)
        for t, (n0, n1, _, _) in enumerate(tiles_all[k]):
            nn = n1 - n0
            base = k * NPC
            sb, st = divmod(t, SPB)
            col = sb * 2 * BLK_N + st * SUB_N
            out[base + n0:base + n1, 0:H] = outT[:, col:col + nn].T
            out[base + n0:base + n1, H:2 * H] = \
                outT[:, col + BLK_N:col + BLK_N + nn].T
    return out
